# revision 12
# baseline (speedup 1.0000x reference)
"""Trainium2 Bass kernel for nn_BatchDifferentiableKF (v2).

Problem: batched 4-state Kalman filter, B=16384 rows, T=512 steps,
state [px, py, vx, vy], measurements = predicted velocities (B, T, 2).

Structure exploited (see kernel_v1 for derivation):
  * Gains are data-independent -> fixed schedule, computed on host.
  * x/y channels decouple into two identical scalar filters, LINEAR in
    (z, p0):  p_t = p_{t-1} + g[t] v_{t-1} + k_p[t] z_t ;
              v_t = a[t] v_{t-1} + k_v[t] z_t.
  * Chunk T into 4 x 128. Per chunk the map z -> (p, v) is a dense
    128x128 matrix pair; chunks 1..3 share steady-state weights.

v2 changes vs v1:
  * Channel-separated matmuls: contract 128 time-steps of ONE channel
    per matmul (N=256 out cols) instead of interleaved (2 x N=512)
    -> half the PE streaming work.
  * Per-channel carry chain in [2, kw] tiles (p, v rows); carry
    injection is one short (N=160) matmul per (chunk, tile, channel).
  * f32->bf16 cast happens inside the input DMA (SWDGE), removing the
    DVE cast pass and the f32 staging buffers.
  * All constants packed into 3 DMAs; PE warm-up runs on a memset tile
    at t~0 so the HAM clock gate opens before real work arrives.
  * Per-tile output DMA right after its last chunk -> short drain tail.

Sharding: embarrassingly parallel over batch across the 8 cores.
"""

import numpy as np
import ml_dtypes

B_FULL = 16384
T = 512
C = 128          # chunk length
NCH = T // C     # 4 chunks
N_CORES = 8
B_CORE = B_FULL // N_CORES   # 2048
VEL_KEEP = 32    # vel carry columns kept (a_ss^32 ~ 5e-19)

bf16 = ml_dtypes.bfloat16


# ----------------------------------------------------------------------------
# Host-side weight construction (float64)
# ----------------------------------------------------------------------------

def _gains(dt, q_pos, q_vel, r_vel, n):
    """Gain schedule k_p[t], k_v[t] of the decoupled scalar filter, P0=I."""
    dt = float(np.float32(dt))
    r = float(np.float32(r_vel)) + float(np.float32(1e-6))
    qp = float(np.float32(q_pos))
    qv = float(np.float32(q_vel))
    Ppp, Ppv, Pvv = 1.0, 0.0, 1.0
    k_p = np.zeros(n)
    k_v = np.zeros(n)
    for t in range(n):
        Ppv_ = Ppv + dt * Pvv
        Ppp_ = Ppp + 2.0 * dt * Ppv + dt * dt * Pvv + qp
        Pvv_ = Pvv + qv
        S = Pvv_ + r
        k_p[t] = Ppv_ / S
        k_v[t] = Pvv_ / S
        Ppp = Ppp_ - k_p[t] * Ppv_
        Ppv = Ppv_ - k_p[t] * Pvv_
        Pvv = Pvv_ - k_v[t] * Pvv_
    return k_p, k_v


def _chunk_maps(k_p, k_v, dt):
    """Per-chunk affine maps (p_in, v_in, z[0..C-1]) -> (p[..], v[..])."""
    g = dt - k_p
    a = 1.0 - k_v
    Wp = np.zeros((NCH, C, C))
    Wv = np.zeros((NCH, C, C))
    Av = np.zeros((NCH, C))
    Bv = np.zeros((NCH, C))
    for m in range(NCH):
        pcoef = np.zeros(C + 1)
        vcoef = np.zeros(C + 1)
        vcoef[0] = 1.0
        for i in range(C):
            t = m * C + i
            pcoef = pcoef + g[t] * vcoef
            pcoef[1 + i] += k_p[t]
            vcoef = a[t] * vcoef
            vcoef[1 + i] += k_v[t]
            Bv[m, i] = pcoef[0]
            Wp[m, i] = pcoef[1:]
            Av[m, i] = vcoef[0]
            Wv[m, i] = vcoef[1:]
    return Wp, Wv, Av, Bv


def build_weights(dt, q_pos, q_vel, r_vel):
    """Device constants.

    cpack1 [128, 644] bf16 = wall(2x256) | bwv(4) | identb(128)
      wall cols [256*ms + i]: pos weights W[j, i] = Wp[ms][i, j] (i = out
      time, j = in time); [256*ms + 128 + i]: vel weights.
      bwv cols (2*ms, 2*ms+1): end-row weights (p, v) for chunk type ms.
    cw2pack [2, 324] bf16, rows (p_in, v_in):
      cols [160*ms : 160*ms+160]: injection weights -> [pos(128) | vel(32)]
      cols [320+2*ms : 322+2*ms]: M(ms)^T, the 2x2 carry-advance transpose.
    """
    dtf = float(np.float32(dt))
    k_p, k_v = _gains(dt, q_pos, q_vel, r_vel, T)
    Wp, Wv, Av, Bv = _chunk_maps(k_p, k_v, dtf)

    cpack1 = np.zeros((128, 644))
    for ms in range(2):
        cpack1[:, 256 * ms + 0:256 * ms + 128] = Wp[ms].T
        cpack1[:, 256 * ms + 128:256 * ms + 256] = Wv[ms].T
        cpack1[:, 512 + 2 * ms] = Wp[ms][C - 1]
        cpack1[:, 513 + 2 * ms] = Wv[ms][C - 1]
    cpack1[:, 516:644] = np.eye(128)

    cw2pack = np.zeros((2, 324))
    for ms in range(2):
        cw2pack[0, 160 * ms:160 * ms + 128] = 1.0
        cw2pack[1, 160 * ms:160 * ms + 128] = Bv[ms]
        cw2pack[1, 160 * ms + 128:160 * ms + 160] = Av[ms][:VEL_KEEP]
        M = np.array([[1.0, Bv[ms][C - 1]], [0.0, Av[ms][C - 1]]])
        cw2pack[:, 320 + 2 * ms:322 + 2 * ms] = M.T
    return {"cpack1": cpack1.astype(bf16), "cw2pack": cw2pack.astype(bf16)}


# ----------------------------------------------------------------------------
# Bass kernel
# ----------------------------------------------------------------------------

def build_nc(n_bt):
    """Build the Bass program for one core processing n_bt*128 batch rows."""
    import concourse.bass as bass
    import concourse.tile as tile
    from concourse import bacc, mybir
    from contextlib import ExitStack

    f32 = mybir.dt.float32
    bf = mybir.dt.bfloat16

    b_sz = n_bt * 128
    JG = 4                     # batch-tiles per group
    kw = 128 * JG              # group width (cols)
    n_jg = n_bt // JG
    nc = bacc.Bacc("TRN2", target_bir_lowering=False, debug=False)

    z_in = nc.dram_tensor("z_in", [b_sz, 1024], f32, kind="ExternalInput").ap()
    p0z_in = [nc.dram_tensor(f"p0z{c}_in", [2, b_sz], bf,
                             kind="ExternalInput").ap() for c in range(2)]
    cpack1_d = nc.dram_tensor("cpack1", [128, 644], bf,
                              kind="ExternalInput").ap()
    cw2pack_d = nc.dram_tensor("cw2pack", [2, 324], bf,
                               kind="ExternalInput").ap()
    pos_out = nc.dram_tensor("pos_out", [b_sz, 1024], f32,
                             kind="ExternalOutput").ap()
    vel_out = nc.dram_tensor("vel_out", [b_sz, 1024], f32,
                             kind="ExternalOutput").ap()

    with tile.TileContext(nc) as tc, ExitStack() as ctx:
        const = ctx.enter_context(tc.tile_pool(name="const", bufs=1))
        ztp = ctx.enter_context(tc.tile_pool(name="ztp", bufs=1))
        zbp = ctx.enter_context(tc.tile_pool(name="zbp", bufs=8))
        stage = ctx.enter_context(tc.tile_pool(name="stage", bufs=2))
        ps_out0 = ctx.enter_context(tc.tile_pool(name="ps_out0", bufs=2,
                                                 space="PSUM"))
        ps_out1 = ctx.enter_context(tc.tile_pool(name="ps_out1", bufs=2,
                                                 space="PSUM"))
        ps_tr = ctx.enter_context(tc.tile_pool(name="ps_tr", bufs=2,
                                               space="PSUM"))
        ps_c = ctx.enter_context(tc.tile_pool(name="ps_c", bufs=2,
                                              space="PSUM"))

        # ---- constants ----
        csb = const.tile([128, 644], bf, name="csb", tag="csb")
        cwsb = const.tile([2, 324], bf, name="cwsb", tag="cwsb")
        p0sb = [const.tile([2, b_sz], bf, name=f"p0sb{c}", tag=f"p0sb{c}")
                for c in range(2)]
        warmw = const.tile([128, 128], bf, name="warmw", tag="warmw")
        nc.scalar.dma_start(csb[:], cpack1_d)
        nc.scalar.dma_start(cwsb[:], cw2pack_d)
        for c in range(2):
            nc.scalar.dma_start(p0sb[c][:], p0z_in[c])
        identb = csb[:, 516:644]

        # ---- PE warm-up on a locally memset tile: starts at t~0 with no
        # DMA dependency, keeps PE busy past the HAM window ----
        nc.vector.memset(warmw[:], 0.03125)
        warm_ps = ps_out0.tile([128, 256], f32, tag="out0")
        for wi in range(40):
            nc.tensor.matmul(warm_ps[:, 0:128], warmw[:], warmw[:],
                             start=(wi == 0), stop=(wi == 39))

        # zt[m] [128 t, 2 c, b] bf16 time-major z
        zt = [ztp.tile([128, 2, b_sz], bf, name=f"zt_{m}", tag=f"zt{m}")
              for m in range(NCH)]

        def load_z(j):
            """z batch-tile j: SWDGE DMA with f32->bf16 cast."""
            zb = zbp.tile([128, 1024], bf, name=f"zb_{j}", tag="zb")
            nc.gpsimd.dma_start(zb[:], z_in[128 * j:128 * (j + 1), :])
            return zb

        tr_idx = [0]

        def emit_transpose(zb, j, m, c):
            bsl = slice(128 * j, 128 * (j + 1))
            src = zb.rearrange("p (t c) -> p c t", c=2)[:, c,
                                                        128 * m:128 * (m + 1)]
            tp = ps_tr.tile([128, 128], bf, name=f"tp_{j}_{m}_{c}", tag="tp")
            nc.tensor.transpose(tp[:], src, identb)
            if tr_idx[0] % 2 == 0:
                nc.vector.tensor_copy(zt[m][:, c, bsl], tp[:])
            else:
                nc.scalar.copy(zt[m][:, c, bsl], tp[:])
            tr_idx[0] += 1

        # ---- main loop ----
        zb_cur = [load_z(jj) for jj in range(JG)]
        for jj in range(JG):
            for m in range(NCH):
                for c in range(2):
                    emit_transpose(zb_cur[jj], jj, m, c)

        for jg in range(n_jg):
            gsl = slice(kw * jg, kw * (jg + 1))
            pending = []
            if jg + 1 < n_jg:
                zb_next = [load_z((jg + 1) * JG + jj) for jj in range(JG)]
                pending = [(zb_next[jj], (jg + 1) * JG + jj, m, c)
                           for jj in range(JG) for m in range(NCH)
                           for c in range(2)]
            pi = 0

            # carry chain: state[c][k] [2, kw] = chunk-(k-1) end state
            # state[c][0] = (p0_c, 0); state[k] = bwv.z_{k-1} + M state[k-1]
            state = [[None] * NCH for _ in range(2)]
            for c in range(2):
                state[c][0] = p0sb[c][:, gsl]

            pos_stage = [None] * JG
            vel_stage = [None] * JG
            for m in range(NCH):
                ms = min(m, 1)
                if m + 1 < NCH:
                    # chain step producing state[m+1]; its ACT copy
                    # completes during this m's main matmuls
                    k = m + 1
                    kms = min(k - 1, 1)
                    for c in range(2):
                        cps = ps_c.tile([2, kw], f32, tag="cps")
                        nc.tensor.matmul(cps[:],
                                         csb[:, 512 + 2 * kms:514 + 2 * kms],
                                         zt[k - 1][:, c, gsl],
                                         start=True, stop=False)
                        nc.tensor.matmul(cps[:],
                                         cwsb[:, 320 + 2 * kms:322 + 2 * kms],
                                         state[c][k - 1],
                                         start=False, stop=True)
                        st = stage.tile([2, kw], bf, name=f"st_{jg}_{k}_{c}",
                                        tag=f"st{k}{c}", bufs=2)
                        nc.scalar.copy(st[:], cps[:])
                        state[c][k] = st[:]
                for jj in range(JG):
                    j = jg * JG + jj
                    bsl = slice(128 * j, 128 * (j + 1))
                    if m == 0:
                        pos_stage[jj] = stage.tile([128, 1024], f32,
                                                   name=f"pos_st_{j}",
                                                   tag=f"pos_st{jj}")
                        vel_stage[jj] = stage.tile([128, 1024], f32,
                                                   name=f"vel_st_{j}",
                                                   tag=f"vel_st{jj}")
                    outp = [None, None]
                    for c in range(2):
                        pool = ps_out0 if c == 0 else ps_out1
                        op = pool.tile([128, 256], f32, tag=f"out{c}")
                        nc.tensor.matmul(op[:], zt[m][:, c, bsl],
                                         csb[:, 256 * ms:256 * ms + 256],
                                         start=True, stop=False)
                        stsl = state[c][m][:, 128 * jj:128 * (jj + 1)]
                        nc.tensor.matmul(
                            op[:, 0:160], stsl,
                            cwsb[:, 160 * ms:160 * ms + 160],
                            start=False, stop=True)
                        outp[c] = op

                    for _ in range(2):
                        if pi < len(pending):
                            emit_transpose(*pending[pi])
                            pi += 1

                    # evacuate: pos rows t' -> cols 256m + 2t' + c
                    pblk = pos_stage[jj][:, 256 * m:256 * (m + 1)]
                    vblk = vel_stage[jj][:, 256 * m:256 * (m + 1)]
                    pblk = pblk.rearrange("p (t c) -> p t c", c=2)
                    vblk = vblk.rearrange("p (t c) -> p t c", c=2)
                    for c in range(2):
                        nc.vector.tensor_copy(pblk[:, :, c],
                                              outp[c][:, 0:128])
                        nc.scalar.copy(vblk[:, :, c], outp[c][:, 128:256])
                    if m == NCH - 1:
                        nc.sync.dma_start(pos_out[bsl, :], pos_stage[jj][:])
                        nc.gpsimd.dma_start(vel_out[bsl, :], vel_stage[jj][:])

            while pi < len(pending):
                emit_transpose(*pending[pi])
                pi += 1

    nc.compile()
    return nc


# ----------------------------------------------------------------------------
# Host entry point
# ----------------------------------------------------------------------------

_CACHE = {}

# test-harness knobs (ignored in normal use)
PROFILE = False
LAST_RESULT = None


def _get_nc(n_bt):
    if n_bt not in _CACHE:
        _CACHE[n_bt] = build_nc(n_bt)
    return _CACHE[n_bt]


def kernel(pred_vel, dt, p0, q_pos, q_vel, r_vel):
    from concourse.bass_utils import run_bass_kernel_spmd

    z = np.ascontiguousarray(np.asarray(pred_vel, dtype=np.float32))
    p0 = np.ascontiguousarray(np.asarray(p0, dtype=np.float32))
    assert z.shape == (B_FULL, T, 2) and p0.shape == (B_FULL, 2)

    weights = build_weights(dt, q_pos, q_vel, r_vel)
    # p0z_c [2, B]: rows (p0 channel c, zeros) -- the initial (p, v) carry
    p0z = [np.zeros((2, B_FULL), dtype=bf16) for _ in range(2)]
    for c in range(2):
        p0z[c][0] = p0[:, c].astype(bf16)
    nc = _get_nc(B_CORE // 128)

    in_maps = []
    for i in range(N_CORES):
        sl = slice(i * B_CORE, (i + 1) * B_CORE)
        m = {"z_in": z[sl].reshape(B_CORE, 2 * T),
             "p0z0_in": np.ascontiguousarray(p0z[0][:, sl]),
             "p0z1_in": np.ascontiguousarray(p0z[1][:, sl])}
        m.update(weights)
        in_maps.append(m)

    res = run_bass_kernel_spmd(nc, in_maps, core_ids=list(range(N_CORES)),
                               trace=PROFILE)
    global LAST_RESULT
    LAST_RESULT = res
    pos = np.concatenate([r["pos_out"].reshape(B_CORE, T, 2)
                          for r in res.results], axis=0)
    vel = np.concatenate([r["vel_out"].reshape(B_CORE, T, 2)
                          for r in res.results], axis=0)
    return pos, vel
